# revision 27
# baseline (speedup 1.0000x reference)
"""Causal multi-head attention (B=4, T=2048, C=1024, 16 heads) on 8 TRN2 NeuronCores.

Sharding: core (b, g) handles batch b and head-group g (8 heads = 512 features).
Each core projects Q/K/V for its own heads only (no duplicated projection work),
runs causal attention for its 8 heads over the full sequence, and computes a
PARTIAL output projection (contraction over its 512 ctx features). The host sums
the two partials per batch and adds the (bv-folded) output bias.

Causality: q-chunks of 512 attend to kv in [0, 512(j+1)); within the diagonal
512x512 block, score/exp/AV work is trimmed at 128 granularity and the
remaining triangular 128x128 blocks are masked with a single {0,1} tile.

Engine budget per core (bf16 matmuls, fp32 PSUM):
  PE  ~180us: QKVO projections (112) + scores/AV head-pair packed (60) + l-sums
  ACT ~163us: exp only (projection epilogues are on DVE instead)
  DVE ~110us: bias epilogues, P accumulation (for softmax denominators),
              masks, 1/l, ctx normalize, PSUM->SBUF copies
Softmax denominators: P tiles are accumulated on DVE into acc[kv,2,q]; a
ones-vector matmul reduces over kv partitions; normalize multiplies ctx by a
gpsimd-broadcast 1/l tile before the output projection.

Emission interleaves projection tiles as PE filler into the ACT-bound
attention stream (scores pipelined one step ahead of AV).
"""

import numpy as np
import ml_dtypes

B, T, C, NH, D = 4, 2048, 1024, 16, 64
P = 128
G = 8                 # heads per core
CH = 512              # q-chunk size
NCH = T // CH         # 4 q-chunks
KC = C // P           # 8 contraction chunks for QKV projections
OC = (C // 2) // P    # 4 contraction chunks for the partial O projection
NHP = G // 2          # 4 head pairs per core

_CACHE = {}


def _build():
    import concourse.bacc as bacc
    import concourse.tile as tile
    import concourse.mybir as mybir
    from concourse.bass import ts, ds

    f32 = mybir.dt.float32
    bf16 = mybir.dt.bfloat16
    f8 = mybir.dt.float8e4
    DR = mybir.MatmulPerfMode.DoubleRow
    EXP = mybir.ActivationFunctionType.Exp
    MUL = mybir.AluOpType.mult
    ADD = mybir.AluOpType.add

    nc = bacc.Bacc("TRN2", target_bir_lowering=False, debug=False, num_devices=8)

    def din(name, shape, dt=bf16):
        return nc.dram_tensor(name, list(shape), dt, kind="ExternalInput").ap()

    xT = din("xT", (C, T), f8)       # x^T for this batch (fp8: Q/K path only)
    xbT = din("xbT", (C, T))         # x^T in bf16 (V path: fp8 V noise does
                                     # not average out on short-context rows)
    wqT = din("wqT", (C, CH), f8)    # (Wq.T/8 * 64) columns for this head group
    wkT = din("wkT", (C, CH), f8)    # Wk.T * 64
    wvT = din("wvT", (C, CH))        # Wv.T (bf16)
    woT = din("woT", (CH, C))        # Wo.T rows for this head group (bf16)
    bq = din("bq", (P, NHP), f32)    # bq/8, chunked per 128-feature block
    bk = din("bk", (P, NHP), f32)
    tri = din("tri", (P, P))         # {0,1}, tri[kv, q] = kv <= q
    out = nc.dram_tensor("out", [C, T], bf16, kind="ExternalOutput").ap()

    x_v = xT.rearrange("(k p) t -> p k t", p=P)      # [128, 8, 2048]
    xb_v = xbT.rearrange("(k p) t -> p k t", p=P)
    wq_v = wqT.rearrange("(k p) m -> p k m", p=P)    # [128, 8, 512]
    wk_v = wkT.rearrange("(k p) m -> p k m", p=P)
    wv_v = wvT.rearrange("(k p) m -> p k m", p=P)
    wo_v = woT.rearrange("(k p) m -> p k m", p=P)    # [128, 4, 1024]

    from contextlib import ExitStack
    with ExitStack() as ctx:
        tc = ctx.enter_context(tile.TileContext(nc))

        consts = ctx.enter_context(tc.tile_pool(name="consts", bufs=1))
        big = ctx.enter_context(tc.tile_pool(name="big", bufs=1))
        ctxpool = ctx.enter_context(tc.tile_pool(name="ctxT", bufs=2))
        ptpool = ctx.enter_context(tc.tile_pool(name="pt", bufs=4))
        accpool = ctx.enter_context(tc.tile_pool(name="acc", bufs=2))
        lrpool = ctx.enter_context(tc.tile_pool(name="lr", bufs=2))
        lbpool = ctx.enter_context(tc.tile_pool(name="lb", bufs=2))
        opool = ctx.enter_context(tc.tile_pool(name="o", bufs=2))
        psumS = ctx.enter_context(tc.tile_pool(name="psumS", bufs=2, space="PSUM"))
        psumC = ctx.enter_context(tc.tile_pool(name="psumC", bufs=2, space="PSUM"))
        psumP = ctx.enter_context(tc.tile_pool(name="psumP", bufs=2, space="PSUM"))

        bq_sb = consts.tile([P, NHP], f32)
        bk_sb = consts.tile([P, NHP], f32)
        tri_sb = consts.tile([P, 1, P], bf16)
        ones_sb = consts.tile([P, 1], bf16)
        warm = consts.tile([1, 2], f32)
        nc.vector.memset(ones_sb[:], 1.0)
        nc.vector.memset(warm[:], 0.0)
        # preload the exp table set early
        nc.scalar.activation(warm[:], warm[:], EXP)

        X = big.tile([P, KC, T], f8)        # fp8 x (Q/K projections)
        XB = big.tile([P, KC, T], bf16)     # bf16 x (V projection)
        WQ = big.tile([P, KC, CH], f8)
        WK = big.tile([P, KC, CH], f8)
        WV = big.tile([P, KC, CH], bf16)
        WO = big.tile([P, OC, C], bf16)
        KT = big.tile([P, NHP, T], bf16)    # K^T  [d(2-head packed), hp, t]
        QT = big.tile([P, NHP, T], bf16)
        V = big.tile([P, T // P, G, D], bf16)  # [kv_local, kv_chunk, head, d]

        # DMA order = first-use order; X slice 0 + WK first (gate the first
        # matmul), weights on the sync queue, bulk X on gpsimd in parallel.
        nc.sync.dma_start(X[:, :, 0:CH], x_v[:, :, 0:CH])
        for hp in range(NHP):
            nc.sync.dma_start(WK[:, :, ts(hp, P)], wk_v[:, :, ts(hp, P)])
        nc.sync.dma_start(bk_sb[:], bk)
        nc.sync.dma_start(bq_sb[:], bq)
        nc.gpsimd.dma_start(XB[:, :, 0:CH], xb_v[:, :, 0:CH])
        nc.sync.dma_start(WV[:], wv_v)
        nc.sync.dma_start(WQ[:], wq_v)
        nc.gpsimd.dma_start(X[:, :, ds(CH, 3 * CH)], x_v[:, :, ds(CH, 3 * CH)])
        for tb in range(1, NCH):
            nc.sync.dma_start(XB[:, :, ds(CH * tb, CH)],
                              xb_v[:, :, ds(CH * tb, CH)])
        nc.sync.dma_start(tri_sb[:, 0, :], tri)
        nc.gpsimd.dma_start(WO[:], wo_v)

        # ---------- projection tile emitters (filler units) ----------
        # All projections are fp8 DoubleRow over k-chunk pairs; weights were
        # pre-scaled by 64 (32 for Wo) on the host, descaled in the epilogue.
        def kt_tile(tb, hp):
            """KT[:, hp, 512*tb : ...] <- (Wk chunk)^T @ x chunk + bk."""
            ps = psumP.tile([P, CH], f32, tag="pp", name=f"pk{tb}{hp}")
            for k in range(0, KC, 2):
                nc.tensor.matmul(ps[:], WK[:, k : k + 2, ts(hp, P)],
                                 X[:, k : k + 2, ds(CH * tb, CH)],
                                 start=(k == 0), stop=(k == KC - 2),
                                 perf_mode=DR)
            nc.vector.tensor_scalar(
                out=KT[:, hp, ds(CH * tb, CH)], in0=ps[:],
                scalar1=1.0 / 64, scalar2=bk_sb[:, hp : hp + 1],
                op0=MUL, op1=ADD)

        def q_tile(j, hp):
            ps = psumP.tile([P, CH], f32, tag="pp", name=f"pq{j}{hp}")
            for k in range(0, KC, 2):
                nc.tensor.matmul(ps[:], WQ[:, k : k + 2, ts(hp, P)],
                                 X[:, k : k + 2, ds(CH * j, CH)],
                                 start=(k == 0), stop=(k == KC - 2),
                                 perf_mode=DR)
            nc.vector.tensor_scalar(
                out=QT[:, hp, ds(CH * j, CH)], in0=ps[:],
                scalar1=1.0 / 64, scalar2=bq_sb[:, hp : hp + 1],
                op0=MUL, op1=ADD)

        def v_tile(i):
            """V rows [128i : 128(i+1)] for all 8 heads (x chunk stationary)."""
            ps = psumP.tile([P, CH], f32, tag="pp", name=f"pv{i}")
            for k in range(KC):
                nc.tensor.matmul(ps[:], XB[:, k, ts(i, P)], WV[:, k, :],
                                 start=(k == 0), stop=(k == KC - 1))
            nc.vector.tensor_copy(V[:, i, :, :],
                                  ps.rearrange("p (h d) -> p h d", d=D))

        def o_tile(j, m, ctxT_j):
            """Partial out rows [128m:...], q chunk j (no bias; host adds it)."""
            ps = psumP.tile([P, CH], f32, tag="pp", name=f"po{j}{m}")
            for k in range(OC):
                nc.tensor.matmul(ps[:], WO[:, k, ts(m, P)], ctxT_j[:, k, :],
                                 start=(k == 0), stop=(k == OC - 1))
            o_sb = opool.tile([P, CH], bf16, tag="o", name=f"o{j}{m}")
            nc.vector.tensor_copy(o_sb[:], ps[:])
            q = nc.gpsimd if m % 2 else nc.sync
            q.dma_start(out[ts(m, P), ds(CH * j, CH)], o_sb[:])

        # ---------- attention ----------
        tri_b = tri_sb[:, 0:1, :].to_broadcast((P, 2, P))

        def attn_pair(hp, j, ctxT_j, fill):
            """Head pair hp, q rows [512j : 512(j+1)], kv in [0, 512(j+1))."""
            nkv = 4 * (j + 1)
            ctx_ps = psumC.tile([P, CH], f32, tag="ctx", name=f"cx{j}{hp}")
            acc = accpool.tile([P, 2, CH], bf16, tag="acc", name=f"ac{j}{hp}")
            pts = {}

            def score_step(c):
                m = c - 4 * j
                qo = P * m if m >= 0 else 0
                st = psumS.tile([P, 2, CH], f32, tag="st", name=f"st{j}{hp}{c}")
                for hh in range(2):
                    nc.tensor.matmul(
                        st[:, hh, qo:],
                        KT[ds(64 * hh, 64), hp, ts(c, P)],
                        QT[ds(64 * hh, 64), hp, ds(CH * j + qo, CH - qo)],
                        start=True, stop=True)
                pt = ptpool.tile([P, 2, CH], bf16, tag="pt", name=f"pt{j}{hp}{c}")
                nc.scalar.activation(pt[:, :, qo:], st[:, :, qo:], EXP)
                if m >= 0:
                    nc.vector.tensor_tensor(pt[:, :, ds(qo, P)],
                                            pt[:, :, ds(qo, P)], tri_b, MUL)
                if c == 0:
                    nc.vector.tensor_copy(acc[:], pt[:])
                else:
                    nc.vector.tensor_tensor(acc[:, :, qo:], acc[:, :, qo:],
                                            pt[:, :, qo:], ADD)
                pts[c] = (pt, qo)

            def av_step(c):
                pt, qo = pts.pop(c)
                for hh in range(2):
                    nc.tensor.matmul(
                        ctx_ps[ds(64 * hh, 64), qo:],
                        V[:, c, 2 * hp + hh, :],
                        pt[:, hh, qo:],
                        start=(c == 0), stop=(c == nkv - 1))

            # scores pipelined one step ahead of AV; fillers paced per step
            # (emitted between the score and AV pairs so filler streaming
            # covers the AV LDWEIGHTS)
            score_step(0)
            for c in range(1, nkv):
                score_step(c)
                fill()
                av_step(c - 1)
            fill()
            av_step(nkv - 1)

            # epilogue: l = colsum(P) via ones-matmul; ctxT = ctx / l
            l_ps = psumS.tile([P, 2, CH], f32, tag="st", name=f"l{j}{hp}")
            nc.tensor.matmul(l_ps[0:1, 0, :], ones_sb[:, 0:1], acc[:, 0, :],
                             start=True, stop=True)
            nc.tensor.matmul(l_ps[0:1, 1, :], ones_sb[:, 0:1], acc[:, 1, :],
                             start=True, stop=True)
            linv = lrpool.tile([1, 2, CH], f32, tag="lr", name=f"li{j}{hp}")
            nc.vector.reciprocal_approx_fast(linv[:, 0, :], l_ps[0:1, 0, :])
            nc.vector.reciprocal_approx_fast(linv[:, 1, :], l_ps[0:1, 1, :])
            for hh in range(2):
                lb = lbpool.tile([P, CH], f32, tag="lb", name=f"lb{j}{hp}{hh}")
                nc.gpsimd.partition_broadcast(lb[:], linv[:, hh, :],
                                              channels=P)
                nc.vector.tensor_tensor(ctxT_j[ds(64 * hh, 64), hp, :],
                                        ctx_ps[ds(64 * hh, 64), :],
                                        lb[ds(64 * hh, 64), :], MUL)

        # ---------- schedule ----------
        # prologue: K/V/Q for kv & q block 0
        for hp in range(NHP):
            kt_tile(0, hp)
        for i in range(NCH):
            v_tile(i)
        for hp in range(NHP):
            q_tile(0, hp)

        ctxT = [None] * NCH
        for j in range(NCH):
            ctxT[j] = ctxpool.tile([P, NHP, CH], bf16, tag="ctxT", name=f"cT{j}")
            # filler units due during attention chunk j. O-projection chunks
            # are deferred to the latest possible slot (chunk 3 is otherwise
            # ACT-bound with an idling, HAM-oscillating PE): o(0) runs during
            # chunk 1, o(1) and o(2) during chunk 3.
            units = []
            if j < NCH - 1:
                units += [lambda hp=hp: q_tile(j + 1, hp) for hp in range(NHP)]
                units += [lambda hp=hp: kt_tile(j + 1, hp) for hp in range(NHP)]
                units += [lambda i=i: v_tile(4 * (j + 1) + i) for i in range(4)]
            ojs = {1: [0], 3: [1, 2]}.get(j, [])
            units += [lambda m=m, jj=jj: o_tile(jj, m, ctxT[jj])
                      for jj in ojs for m in range(KC)]
            steps = NHP * 4 * (j + 1)
            state = {"s": 0, "f": 0}

            def fill(units=units, steps=steps, state=state):
                state["s"] += 1
                want = (len(units) * state["s"] + steps - 1) // steps
                while state["f"] < want and state["f"] < len(units):
                    units[state["f"]]()
                    state["f"] += 1

            for hp in range(NHP):
                attn_pair(hp, j, ctxT[j], fill)
            while state["f"] < len(units):
                units[state["f"]]()
                state["f"] += 1

        for m in range(KC):
            o_tile(NCH - 1, m, ctxT[NCH - 1])

    nc.compile()
    return nc


def _shard_inputs(x, Wq, bq, bk_, bv, bo, WqT, WkT, WvT, WoT):
    """Build the 8 per-core input maps. WqT is Wq.T/8; others are plain .T.

    Data tensors go to the device in fp8e4m3; weights are pre-scaled by 64
    (32 for Wo) to clear the e4m3 denormal range, descaled on-device."""
    bf = ml_dtypes.bfloat16
    f8 = ml_dtypes.float8_e4m3
    tri = np.triu(np.ones((P, P), np.float32)).astype(bf)
    in_maps = []
    for b in range(B):
        xTb = np.ascontiguousarray(x[b].T)
        xT8 = xTb.astype(f8)
        xT16 = xTb.astype(bf)
        for g in range(2):
            sl = slice(CH * g, CH * (g + 1))
            in_maps.append({
                "xT": xT8,
                "xbT": xT16,
                "wqT": np.ascontiguousarray(WqT[:, sl] * 64.0).astype(f8),
                "wkT": np.ascontiguousarray(WkT[:, sl] * 64.0).astype(f8),
                "wvT": np.ascontiguousarray(WvT[:, sl]).astype(bf),
                "woT": np.ascontiguousarray(WoT[sl, :]).astype(bf),
                "bq": np.ascontiguousarray((bq[sl] / 8.0).reshape(NHP, P).T),
                "bk": np.ascontiguousarray(bk_[sl].reshape(NHP, P).T),
                "tri": tri,
            })
    return in_maps


def kernel(x, Wq, bq, Wk, bk, Wv, bv, Wo, bo):
    from concourse.bass_utils import run_bass_kernel_spmd

    x = np.asarray(x, np.float32)
    Wq = np.asarray(Wq, np.float32); bq = np.asarray(bq, np.float32)
    Wk = np.asarray(Wk, np.float32); bk = np.asarray(bk, np.float32)
    Wv = np.asarray(Wv, np.float32); bv = np.asarray(bv, np.float32)
    Wo = np.asarray(Wo, np.float32); bo = np.asarray(bo, np.float32)

    if "nc" not in _CACHE:
        _CACHE["nc"] = _build()
    nc = _CACHE["nc"]

    WqT = np.ascontiguousarray(Wq.T / 8.0)
    WkT = np.ascontiguousarray(Wk.T)
    WvT = np.ascontiguousarray(Wv.T)
    WoT = np.ascontiguousarray(Wo.T)
    in_maps = _shard_inputs(x, Wq, bq, bk, bv, bo, WqT, WkT, WvT, WoT)

    res = run_bass_kernel_spmd(nc, in_maps, core_ids=list(range(8)))
    bo_eff = (bo + Wo @ bv).astype(np.float32)
    outf = np.empty((B, T, C), np.float32)
    for b in range(B):
        o = (res.results[2 * b]["out"].astype(np.float32)
             + res.results[2 * b + 1]["out"].astype(np.float32))  # (C, T)
        outf[b] = o.T + bo_eff
    return outf
